# revision 48
# baseline (speedup 1.0000x reference)
"""Deformable bilinear sampling kernel for TRN2 (8-core SPMD).

Per (n,o) pair each output pixel (h,w) needs the 2x2x32c patch at
(h+floor(off_h), w+floor(off_w)) with bilinear corner weights. The host stages
a patch-replicated DRAM table in bf16 packed as int64 rows (256B = 4 corners x
32ch); the device computes int16 gather indices from the offsets, pulls one
row per pixel with gpsimd.dma_gather (int64 view -> 4x cheaper than f32 in
elements), multiplies by j-paired bf16 corner weights (2x DVE mode via a
packed innermost pair dim), sums the 4 corners on the PE via identity-matmul
PSUM accumulation, and evicts psum->sbuf bf16 on ACT before the output DMA.
The per-chunk multiply is split between DVE and Pool so both stay busy.
"""

import numpy as np

import concourse.bacc as bacc
import concourse.bass as bass
import concourse.mybir as mybir
from concourse.library_config import mlp

PAIRS = 4          # (n,o) pairs per core
H = W = 128
C = 32
PAD = 8
HP = 144           # padded anchor grid
NROWS = HP * HP    # 20736 patch rows per pair
NIDX = H * W       # 16384 gathered pixels per pair

F32 = mybir.dt.float32
BF16 = mybir.dt.bfloat16
I16 = mybir.dt.int16
I64 = mybir.dt.int64
OP = mybir.AluOpType
TWO23 = 12582912.0  # 1.5 * 2^23: forces round-to-integer in f32 for |x| < 2^22

# default geometry / assignment knobs (chunk index -> engine)
CH_DEF = 8
MUL_ENG = ["p" if s in (12, 18, 24, 29) else "v" for s in range(32)]
EVICT_ENG = ["a"] * 31 + ["v"]                         # psum -> sbuf bf16


def build_nc(mul_eng=None, evict_eng=None, ch=CH_DEF, ng=12, nx=8, na=8, g64=False, dr=2):
    nchunk = PAIRS * ch
    wch = W // ch              # w-columns per chunk
    nidx_ch = NIDX // ch       # pixels per chunk
    mcol = wch * C             # psum columns per chunk (f32)
    nq = mcol // 512           # psum bank quarters per chunk
    mul_eng = mul_eng or (MUL_ENG if ch == CH_DEF else ["v"] * nchunk)
    evict_eng = evict_eng or (EVICT_ENG if ch == CH_DEF else ["a"] * nchunk)
    assert len(mul_eng) == nchunk and len(evict_eng) == nchunk

    nc = bacc.Bacc("TRN2")
    patches = nc.declare_dram_parameter("patches", [PAIRS, NROWS, 2 * C], mybir.dt.uint32, isOutput=False)
    offn = nc.declare_dram_parameter("offn", [PAIRS, 2, H, W], F32, isOutput=False)
    basen = nc.declare_dram_parameter("basen", [H, W], F32, isOutput=False)
    iden = nc.declare_dram_parameter("iden", [128, 128], BF16, isOutput=False)
    zer16 = nc.declare_dram_parameter("zer16", [128, 1024], I16, isOutput=False)
    out = nc.declare_dram_parameter("out", [PAIRS, H, W, C], BF16, isOutput=True)

    from contextlib import ExitStack

    with ExitStack() as stack:
        ec = stack.enter_context
        block = ec(nc.Block())
        gdt, gcols = (I64, C) if g64 else (mybir.dt.uint32, 2 * C)
        Gb = [ec(nc.sbuf_tensor(f"G{i}", [128, wch, gcols], gdt)) for i in range(ng)]
        Xb = [ec(nc.sbuf_tensor(f"X{i}", [128, wch, 4 * C], BF16)) for i in range(nx)]
        Ab = [ec(nc.sbuf_tensor(f"A{i}", [128, wch, C], BF16)) for i in range(na)]
        Pb = [ec(nc.psum_tensor(f"P{i}", [128, mcol], F32)) for i in range(2)]
        idn = ec(nc.sbuf_tensor("idn", [128, 128], BF16))
        bnat = ec(nc.sbuf_tensor("bnat", [128, W], F32))
        onb = [ec(nc.sbuf_tensor(f"on{i}", [128, 2, W], F32)) for i in range(2)]
        flb = [ec(nc.sbuf_tensor(f"fl{i}", [128, 2, W], F32)) for i in range(2)]
        sfb = [ec(nc.sbuf_tensor(f"sf{i}", [128, 2, W], F32)) for i in range(2)]
        sgb = [ec(nc.sbuf_tensor(f"sg{i}", [128, 2, W], F32)) for i in range(2)]
        wtdb = [ec(nc.sbuf_tensor(f"wtd{i}", [128, 4, W, 2], BF16)) for i in range(PAIRS)]
        tDb = [ec(nc.sbuf_tensor(f"tD{i}", [128, W], F32)) for i in range(PAIRS)]
        dnatb = [ec(nc.sbuf_tensor(f"dnat{i}", [128, W], I16)) for i in range(PAIRS)]
        db = [ec(nc.sbuf_tensor(f"d{i}", [128, 1024], I16)) for i in range(PAIRS)]

        s_inb = ec(nc.semaphore("s_inb"))    # bnat+iden DMAs
        s_zp = [ec(nc.semaphore(f"s_z{i}")) for i in range(PAIRS)]   # d zeroing
        s_in = [ec(nc.semaphore(f"s_in{i}")) for i in range(2)]      # offn loads
        s_sf = ec(nc.semaphore("s_sf"))      # frac done (1/pair)
        s_dn = ec(nc.semaphore("s_dn"))      # dnat ready (1/pair)
        s_wtd = ec(nc.semaphore("s_wtd"))    # weights ready (1/pair)
        s_dw = ec(nc.semaphore("s_dw"))      # wrap DMA progress (80/pair)
        s_wev = ec(nc.semaphore("s_wev"))    # wrap even-copy progress (4/pair)

        def wait_evens(eng, p):
            eng.wait_ge(s_wev, 4 * (p + 1))
        nsem = min(nchunk, 8)
        s_g_ = [ec(nc.semaphore(f"s_g{i}")) for i in range(nsem)]
        s_x_ = [ec(nc.semaphore(f"s_x{i}")) for i in range(nsem)]
        s_mm_ = [ec(nc.semaphore(f"s_mm{i}")) for i in range(nsem)]
        s_ev_ = [ec(nc.semaphore(f"s_ev{i}")) for i in range(nsem)]
        s_out_ = [ec(nc.semaphore(f"s_out{i}")) for i in range(nsem)]

        class CycSem:
            """Per-chunk semaphore view over a cyclic group: chunk s maps to
            sem s%nsem with target value scaled by s//nsem+1."""

            def __init__(self, sems, unit):
                self.sems, self.unit = sems, unit

            def sem(self, s):
                return self.sems[s % nsem]

            def val(self, s):
                return self.unit * (s // nsem + 1)

            def wait(self, eng, s):
                eng.wait_ge(self.sems[s % nsem], self.val(s))

        s_g = CycSem(s_g_, 16)
        s_x = CycSem(s_x_, 1)
        s_mm = CycSem(s_mm_, 1)
        s_ev = CycSem(s_ev_, 1)
        s_out = CycSem(s_out_, 16)
        s_cv = ec(nc.semaphore("s_cv"))      # DVE chain
        s_cp = ec(nc.semaphore("s_cp"))      # Pool chain
        s_ca = ec(nc.semaphore("s_ca"))      # ACT chain

        class Chain:
            """In-order execution on one engine via a chain semaphore. Each op
            either incs the chain sem, or (final=(sem, inc, waitval)) incs that
            sem instead; the next chained op then waits for it."""

            def __init__(self, eng, sem):
                self.eng, self.sem, self.n = eng, sem, 0
                self.extra = []

            def run(self, thunk, final=None):
                if self.n:
                    self.eng.wait_ge(self.sem, self.n)
                for sem, val in self.extra:
                    self.eng.wait_ge(sem, val)
                self.extra = []
                inst = thunk()
                if final is None:
                    inst.then_inc(self.sem, 1)
                    self.n += 1
                else:
                    sem, inc, waitval = final
                    inst.then_inc(sem, inc)
                    if waitval is not None:
                        self.extra.append((sem, waitval))
                return inst

        def psum_view(s):
            return Pb[s % 2][:, :].rearrange("p (w c) -> p w c", c=C)

        def emit_evict(eng, r, s):
            s_mm.wait(eng, s)
            if s >= na:
                s_out.wait(eng, s - na)
            x0 = Xb[s % nx][:, :, 0:C]
            r(lambda s=s, x0=x0: eng.tensor_add(Ab[s % na][:, :, :], psum_view(s), x0),
              final=(s_ev.sem(s), 1, None))

        def emit_mul(eng, r, s):
            p, c = divmod(s, ch)
            s_g.wait(eng, s)
            eng.wait_ge(s_wtd, p + 1)
            if s >= nx:
                s_mm.wait(eng, s - nx)   # PE done reading X[s%nx]
                if evict_eng[s - nx] != "a":
                    s_ev.wait(eng, s - nx)   # evict read X0[s%nx]
            gb = Gb[s % ng][:, :, :].bitcast(BF16)
            ws = c * wch
            for k in range(4):
                xk = Xb[s % nx][:, :, C * k:C * (k + 1)] \
                    .rearrange("p w (c2 j) -> p w c2 j", j=2)
                gk = gb[:, :, C * k:C * (k + 1)].rearrange("p w (c2 j) -> p w c2 j", j=2)
                wk = wtdb[p][:, k, ws:ws + wch, :].unsqueeze(2) \
                    .broadcast_to([128, wch, C // 2, 2])
                r(lambda xk=xk, gk=gk, wk=wk: eng.tensor_mul(xk, gk, wk),
                  final=(s_x.sem(s), 1, s_x.val(s)) if k == 3 else None)

        @block.sync
        def _(sync: bass.BassEngine):
            sync.dma_start(onb[0][:, :, :],
                           offn[0, :, :, :].transpose([1, 0, 2])).then_inc(s_in[0], 16)
            sync.dma_start(bnat[:, :], basen[:, :]).then_inc(s_inb, 16)
            sync.dma_start(db[0][:, :], zer16[:, :]).then_inc(s_zp[0], 16)
            sync.dma_start(idn[:, :], iden[:, :]).then_inc(s_inb, 16)
            if PAIRS > 1:
                sync.dma_start(onb[1][:, :, :],
                               offn[1, :, :, :].transpose([1, 0, 2])).then_inc(s_in[1], 16)
            for i in range(1, PAIRS):
                sync.dma_start(db[i][:, :], zer16[:, :]).then_inc(s_zp[i], 16)
            for pp in range(2, PAIRS):
                sync.wait_ge(s_sf, pp - 1)
                sync.dma_start(onb[pp % 2][:, :, :],
                               offn[pp, :, :, :].transpose([1, 0, 2])).then_inc(s_in[pp % 2], 16)
            for s in range(nchunk):
                p, c = divmod(s, ch)
                s_ev.wait(sync, s)
                dst = out[p, :, c * wch:(c + 1) * wch, :]
                sync.dma_start(dst, Ab[s % na][:, :, :]).then_inc(s_out.sem(s), 16)

        @block.vector
        def _(vector: bass.BassEngine):
            chv = Chain(vector, s_cv)
            r = chv.run

            def pair_math(p):
                st = p % 2
                onf = onb[st][:, :, :]
                fl, sf = flb[st], sfb[st]
                vector.wait_ge(s_in[st], 16 * (p // 2 + 1))
                if p >= 2:
                    # sf/fl parity reuse: Pool weights of pair p-2 done
                    vector.wait_ge(s_wtd, p - 1)
                # floors via round(x - 0.5): a tie (integer offset) yields
                # frac 1.0 with the lower anchor = same interpolated value.
                # (x - 0.5 first: TWO23-0.5 is not representable in f32)
                r(lambda: vector.tensor_scalar(fl[:, :, :], onf, 0.5, TWO23, OP.subtract, OP.add))
                r(lambda: vector.tensor_scalar(fl[:, :, :], fl[:, :, :], 1.0, -TWO23, OP.mult, OP.add))
                r(lambda: vector.tensor_sub(sf[:, :, :], onf, fl[:, :, :]),
                  final=(s_sf, 1, p + 1))
                if p == 0:
                    vector.wait_ge(s_inb, 32)   # bnat (+iden, same queue)
                r(lambda: vector.scalar_tensor_tensor(tDb[p][:, :], fl[:, 0, :], float(HP), fl[:, 1, :], OP.mult, OP.add))
                r(lambda: vector.tensor_add(tDb[p][:, :], tDb[p][:, :], bnat[:, :]))
                r(lambda: vector.tensor_copy(dnatb[p][:, :], tDb[p][:, :]),
                  final=(s_dn, 1, p + 1))
                vector.wait_ge(s_zp[p], 16)
                dwrap = db[p][:, :].rearrange("q (w k) -> q w k", k=8)
                for k in range(0, 8, 2):
                    r(lambda k=k, dwrap=dwrap, p=p: vector.tensor_copy(
                        dwrap[0:16, :, k], dnatb[p][16 * k:16 * (k + 1), :]),
                      final=(s_wev, 1, None))

            for p in range(PAIRS):
                pair_math(p)
            for s in range(nchunk):
                if mul_eng[s] == "v":
                    emit_mul(vector, r, s)
                if evict_eng[s] == "v":
                    emit_evict(vector, r, s)

        @block.scalar
        def _(act: bass.BassEngine):
            cha = Chain(act, s_ca)
            ra = cha.run
            for p in range(PAIRS):
                act.wait_ge(s_dn, p + 1)
                act.wait_ge(s_zp[p], 16)
                if p >= 1:
                    act.wait_ge(s_dw, 80 * p)   # drain prior pair's wrap DMAs
                dwrap = db[p][:, :].rearrange("q (w k) -> q w k", k=8)
                with nc.allow_non_contiguous_dma(reason="4KB idx-wrap strided dst"):
                    for k in range(1, 8, 2):
                        act.dma_start(dwrap[0:16, :, k],
                                      dnatb[p][16 * k:16 * (k + 1), :]).then_inc(s_dw, 16)
                # replicate wrapped indices to partitions 16-31 (the group the
                # Q7 descriptor-gen core reads on HW)
                act.wait_ge(s_dw, 80 * p + 64)
                wait_evens(act, p)
                act.dma_start(db[p][16:32, :], db[p][0:16, :]).then_inc(s_dw, 16)
            for s in range(nchunk):
                if evict_eng[s] == "a":
                    s_mm.wait(act, s)
                    if s >= na:
                        s_out.wait(act, s - na)
                    ra(lambda s=s: act.copy(Ab[s % na][:, :, :], psum_view(s)),
                       final=(s_ev.sem(s), 1, None))

        @block.gpsimd
        def _(gpsimd: bass.BassGpSimd):
            chp = Chain(gpsimd, s_cp)
            rp = chp.run
            gpsimd.load_library(mlp)

            def pair_weights(p):
                st = p % 2
                sf, sg, wtd = sfb[st], sgb[st], wtdb[p]
                gpsimd.wait_ge(s_sf, p + 1)
                rp(lambda st=st: gpsimd.tensor_scalar(sgb[st][:, :, :], sfb[st][:, :, :], -1.0, 1.0, OP.mult, OP.add))
                ww = [(sg, 0, sg, 1), (sg, 0, sf, 1), (sf, 0, sg, 1), (sf, 0, sf, 1)]
                for k, (ta, ia, tb, ib) in enumerate(ww):
                    a = ta[:, ia, :].unsqueeze(2).broadcast_to([128, W, 2])
                    b = tb[:, ib, :].unsqueeze(2).broadcast_to([128, W, 2])
                    rp(lambda a=a, b=b, k=k, wtd=wtd: gpsimd.tensor_mul(wtd[:, k, :, :], a, b),
                       final=(s_wtd, 1, p + 1) if k == 3 else None)


            for p in range(min(2, PAIRS)):
                pair_weights(p)
            pool_muls = [s for s in range(nchunk) if mul_eng[s] == "p"]
            pending = []   # deferred pool-mul corner ops, interleaved

            def drain(nops):
                while pending and nops > 0:
                    pending.pop(0)()
                    nops -= 1

            def queue_mul(s):
                p, c = divmod(s, ch)

                def first():
                    s_g.wait(gpsimd, s)
                    gpsimd.wait_ge(s_wtd, p + 1)
                    if s >= nx:
                        s_mm.wait(gpsimd, s - nx)
                        if evict_eng[s - nx] != "a":
                            s_ev.wait(gpsimd, s - nx)

                gb = Gb[s % ng][:, :, :].bitcast(BF16)
                ws = c * wch
                for k in range(4):
                    xk = Xb[s % nx][:, :, C * k:C * (k + 1)] \
                        .rearrange("p w (c2 j) -> p w c2 j", j=2)
                    gk = gb[:, :, C * k:C * (k + 1)].rearrange("p w (c2 j) -> p w c2 j", j=2)
                    wk = wtdb[p][:, k, ws:ws + wch, :].unsqueeze(2) \
                        .broadcast_to([128, wch, C // 2, 2])

                    def op(k=k, xk=xk, gk=gk, wk=wk, s=s):
                        if k == 0:
                            first()
                        rp(lambda: gpsimd.tensor_mul(xk, gk, wk),
                           final=(s_x.sem(s), 1, s_x.val(s)) if k == 3 else None)
                    pending.append(op)

            for s in range(nchunk):
                p, c = divmod(s, ch)
                gpsimd.wait_ge(s_dw, 80 * (p + 1))
                wait_evens(gpsimd, p)
                if s >= ng:
                    s_x.wait(gpsimd, s - ng)   # G[s%ng] free
                src_ap = patches[p, :, :].bitcast(I64) if g64 else patches[p, :, :]
                rp(lambda s=s, p=p, c=c, src_ap=src_ap: gpsimd.dma_gather(
                    Gb[s % ng][:, :, :],
                    src_ap,
                    db[p][:, c * (nidx_ch // 16):(c + 1) * (nidx_ch // 16)],
                    nidx_ch,
                    nidx_ch,
                    gcols,
                    single_packet=False,
                ), final=(s_g.sem(s), 16, s_g.val(s)))
                if c == ch - 1 and p + 2 < PAIRS:
                    pair_weights(p + 2)
                if mul_eng[s] == "p":
                    queue_mul(s)
                drain(dr)
            drain(len(pending))
            for s in range(nchunk):
                if evict_eng[s] == "p":
                    emit_evict(gpsimd, rp, s)

        @block.tensor
        def _(tensor: bass.BassEngine):
            tensor.wait_ge(s_inb, 32)
            for s in range(nchunk):
                # corners 1..3 accumulate into psum; corner 0 joins at the
                # eviction add. ACT evicts can't add -> all 4 corners on PE.
                corners = list(range(4)) if evict_eng[s] == "a" else list(range(1, 4))
                s_x.wait(tensor, s)
                if s >= 2:
                    s_ev.wait(tensor, s - 2)   # P[s%2] free
                X = Xb[s % nx]
                P = Pb[s % 2]
                nk = len(corners)
                wq = 512 // C   # w-columns per psum bank quarter
                for q in range(nq):
                    for j, k in enumerate(corners):
                        rhs = X[:, wq * q:wq * (q + 1), C * k:C * (k + 1)]
                        inst = tensor.matmul(
                            P[:, 512 * q:512 * (q + 1)].rearrange("p (w c) -> p w c", c=C),
                            idn[:, :], rhs,
                            start=(j == 0), stop=(j == nk - 1), is_transpose=False)
                        if q == nq - 1 and j == nk - 1:
                            inst.then_inc(s_mm.sem(s), 1)

    nc.compile()
    return nc


# ---------------- host-side helpers ----------------

def build_patches_all(imgs_pairs):
    """imgs_pairs: (NPAIR, C, H, W) f32 -> (NPAIR, NROWS, C) int64 (bf16x4)"""
    import ml_dtypes
    npair = imgs_pairs.shape[0]
    hw_c = np.ascontiguousarray(np.transpose(imgs_pairs, (0, 2, 3, 1)))  # (P,H,W,C)
    padded = np.zeros((npair, HP + 1, HP + 1, C), np.float32)
    padded[:, PAD:PAD + H, PAD:PAD + W] = hw_c
    P = np.empty((npair, HP, HP, 4, C), ml_dtypes.bfloat16)
    P[:, :, :, 0] = padded[:, 0:HP, 0:HP]
    P[:, :, :, 1] = padded[:, 0:HP, 1:HP + 1]
    P[:, :, :, 2] = padded[:, 1:HP + 1, 0:HP]
    P[:, :, :, 3] = padded[:, 1:HP + 1, 1:HP + 1]
    return P.reshape(npair, NROWS, 4 * C).view(np.uint32)


def base_natural():
    h = np.arange(H).reshape(H, 1)
    w = np.arange(W).reshape(1, W)
    return ((h + PAD) * HP + (w + PAD)).astype(np.float32)


def make_in_map(imgs_pairs, offp):
    import ml_dtypes
    return {
        "patches": build_patches_all(imgs_pairs),
        "offn": np.ascontiguousarray(offp),
        "basen": base_natural(),
        "iden": np.eye(128, dtype=ml_dtypes.bfloat16),
        "zer16": np.zeros((128, 1024), np.int16),
    }


# ---------------- public entry point ----------------

N_CORES = 8
PAIRS_TOTAL = 32

LAST_EXEC_TIME_NS = None


def kernel(images, offsets):
    """images (4,8,32,128,128) f32; offsets (4,16,128,128) f32 ->
    (4,8,32,128,128) f32 deformable bilinear sampling, on 8 NeuronCores."""
    import os
    import ml_dtypes
    global LAST_EXEC_TIME_NS
    from concourse.bass_utils import run_bass_kernel_spmd

    images = np.ascontiguousarray(np.asarray(images, dtype=np.float32))
    offsets = np.ascontiguousarray(np.asarray(offsets, dtype=np.float32))
    imgs = images.reshape(PAIRS_TOTAL, C, H, W)
    offp = offsets.reshape(4, 8, 2, H, W).reshape(PAIRS_TOTAL, 2, H, W)

    nc = build_nc()
    in_maps = []
    for core in range(N_CORES):
        sl = slice(core * PAIRS, (core + 1) * PAIRS)
        in_maps.append(make_in_map(imgs[sl], offp[sl]))
    trace = bool(os.environ.get("DK_TRACE"))
    res = run_bass_kernel_spmd(nc, in_maps, list(range(N_CORES)), trace=trace)
    if trace:
        LAST_EXEC_TIME_NS = res.exec_time_ns
        if res.instructions_and_trace:
            print("trace path:", res.instructions_and_trace[1])
    outs = []
    for i in range(N_CORES):
        o = np.asarray(res.results[i]["out"])
        if o.dtype == np.uint16:
            o = o.view(ml_dtypes.bfloat16)
        outs.append(np.asarray(o, dtype=np.float32))
    full = np.concatenate(outs, axis=0)              # (32, H, W, C)
    full = np.transpose(full, (0, 3, 1, 2))          # (32, C, H, W)
    return np.ascontiguousarray(full.reshape(4, 8, C, H, W)).astype(np.float32)
